# revision 26
# baseline (speedup 1.0000x reference)
"""Multi-head self-attention block (B=2, S=2048, D=1024, H=16) on 8 TRN2 cores.

Sharding: 2-way data-parallel over batch x 4-way tensor-parallel over heads.
Core c handles batch b=c//4 with group rank g=c%4 (heads 4g..4g+4). The
out-projection partials are combined with one bf16 ReduceScatter per
q-quarter over the 4-core batch group, so core g owns output rows
[512q + 128g, 512q + 128(g+1)) for q in 0..4 — collectives overlap the
remaining attention quarters instead of forming a serial tail.

Score matmuls are row-tiled: the contraction is only A=64, so two
concurrent 64-row PE tiles (tile_position (0,0)/(64,0)) each compute a
full 128-key x 512-query score block — the array runs at ~2x the naive
block-diagonal packing. K is stored [64 partitions][head][kb-pair][key]
with even key-blocks on partitions 0-63 and odd on 64-127.

QKV runs kt-outer across 8 PSUM banks (scoped pool) so the first matmul
only waits for the first 128-row chunk of W_qkv/embT instead of the
full 5MB load.

Self-contained: hardcodes all shapes; builds the Bass program once.
"""

import os
import sys

sys.path.insert(0, "/opt/trn_rl_repo")

import numpy as np
import ml_dtypes

import concourse.bass as bass
import concourse.tile as tile
from concourse import bacc, mybir
from concourse.bass_utils import run_bass_kernel_spmd

B, S, D, H = 2, 2048, 1024, 16
A = D // H  # 64
NCORES = 8
G = 4  # cores per batch group
HL = H // G  # local heads per core = 4
M_QK = 2 * HL * A  # 512 rows of Q_T+K_T per core
QB = S // G  # 512
EPS = 1e-3
GROUPS = [[0, 1, 2, 3], [4, 5, 6, 7]]

f32 = mybir.dt.float32
f32r = mybir.dt.float32r
bf16 = mybir.dt.bfloat16
i16dt = mybir.dt.int16

AF = mybir.ActivationFunctionType
OP = mybir.AluOpType

# First N_SCH kb-pairs of every unit use the fused DVE Schraudolph
# softmax (mask rows hold 16256 / -65536); the rest use ScalarE exp with
# a 1/0 mask multiply. Balances softmax work across DVE and ScalarE.
N_SCH = 2

_CACHE = {}


def _build():
    nc = bacc.Bacc("TRN2", target_bir_lowering=False, debug=False, num_devices=NCORES)

    # ---- I/O ----
    embT_d = nc.dram_tensor("embT", [D, S], bf16, kind="ExternalInput")
    embres_d = nc.dram_tensor("embres", [QB, D], f32, kind="ExternalInput")
    maskT_d = nc.dram_tensor("maskT", [S, S], bf16, kind="ExternalInput")
    wqk_d = nc.dram_tensor("wqk", [D, M_QK], bf16, kind="ExternalInput")
    wv_d = nc.dram_tensor("wv", [D, HL * A], bf16, kind="ExternalInput")
    bqk_d = nc.dram_tensor("bqk", [128, 4], f32, kind="ExternalInput")
    bv_d = nc.dram_tensor("bv", [1, HL * A], bf16, kind="ExternalInput")
    onesb_d = nc.dram_tensor("onesb", [1, 128], bf16, kind="ExternalInput")
    wout_d = nc.dram_tensor("wout", [128, 2, D], bf16, kind="ExternalInput")
    gamma_d = nc.dram_tensor("gamma", [1, D], bf16, kind="ExternalInput")
    beta_d = nc.dram_tensor("beta", [1, D], bf16, kind="ExternalInput")
    out_d = nc.dram_tensor("out", [QB, D], bf16, kind="ExternalOutput")

    debug = bool(os.environ.get("KERNEL_DEBUG_DUMP"))
    if debug:
        dbg_q2_d = nc.dram_tensor("dbg_q2", [128, HL, S], bf16, kind="ExternalOutput")
        dbg_k_d = nc.dram_tensor("dbg_k", [128, HL, 8, 128], bf16, kind="ExternalOutput")
        dbg_v_d = nc.dram_tensor("dbg_v", [128, 16, HL, 1 + A], bf16, kind="ExternalOutput")
        dbg_mq_d = nc.dram_tensor("dbg_mq", [128, 16, 512], bf16, kind="ExternalOutput")
        dbg_p0_d = nc.dram_tensor("dbg_p0", [128, 16, 512], bf16, kind="ExternalOutput")
        dbg_xT_d = nc.dram_tensor("dbg_xT", [128, 2, S], bf16, kind="ExternalOutput")
        dbg_psx_d = nc.dram_tensor("dbg_psx", [65, 512], f32, kind="ExternalOutput")
        dbg_rcp_d = nc.dram_tensor("dbg_rcp", [1, 512], f32, kind="ExternalOutput")
        dbg_rb_d = nc.dram_tensor("dbg_rb", [64, 512], f32, kind="ExternalOutput")

    with tile.TileContext(nc) as tc:
        with (
            tc.tile_pool(name="big", bufs=1) as big,
            tc.tile_pool(name="persist", bufs=1) as persist,
            tc.tile_pool(name="probs", bufs=2) as probsp,
            tc.tile_pool(name="work", bufs=2) as work,
            tc.tile_pool(name="erp", bufs=2) as erp,
            tc.tile_pool(name="dram", bufs=1, space="DRAM") as dram,
        ):
            # ---------- interleaved W/embT chunk loads: QKV gates on chunk 0 ----------
            embT_sb = big.tile([128, 8, S], bf16, tag="bigslot")
            wqk_sb = persist.tile([128, 8, M_QK], bf16)
            for kt in range(8):
                nc.sync.dma_start(out=wqk_sb[:, kt, :], in_=wqk_d[kt * 128 : (kt + 1) * 128, :])
                nc.sync.dma_start(out=embT_sb[:, kt, :], in_=embT_d[kt * 128 : (kt + 1) * 128, :])
            bqk_sb = persist.tile([128, 4], f32)
            nc.sync.dma_start(out=bqk_sb, in_=bqk_d[:, :])

            wv_sb = persist.tile([128, 8, HL * A], bf16)
            for kt in range(8):
                nc.sync.dma_start(out=wv_sb[:, kt, :], in_=wv_d[kt * 128 : (kt + 1) * 128, :])
            bv_sb = persist.tile([1, HL * A], bf16)
            nc.sync.dma_start(out=bv_sb, in_=bv_d[:, :])
            ones_b = persist.tile([1, 128], bf16)
            nc.sync.dma_start(out=ones_b, in_=onesb_d[:, :])
            wout_sb = persist.tile([128, 2, D], bf16)
            nc.sync.dma_start(out=wout_sb, in_=wout_d[:, :, :])
            eps_sb = persist.tile([128, 1], f32)
            nc.vector.memset(eps_sb, EPS)
            gammabc = persist.tile([128, D], bf16)
            betabc = persist.tile([128, D], bf16)
            for t, dr in ((gammabc, gamma_d), (betabc, beta_d)):
                src = dr[:, :]
                bc = bass.AP(tensor=src.tensor, offset=src.offset, ap=[[0, 128], src.ap[1]])
                nc.sync.dma_start(out=t[:], in_=bc)

            # ---------- QKV projection (kt-outer over 8 PSUM banks) ----------
            # Q duplicated on both partition halves: q2[p, h, s], p<64 and
            # p>=64 both hold Q_h[p % 64, s].
            q2_sb = persist.tile([128, HL, S], bf16)
            # K row-tiled: k_sb[0:64, h, j, m] = K_h[a, keys of kb 2j],
            #              k_sb[64:128, h, j, m] = K_h[a, keys of kb 2j+1].
            k_sb = persist.tile([128, HL, 8, 128], bf16)

            with tc.tile_pool(name="psQK", bufs=2, space="PSUM") as psQK:
                # one mc (4 sc-chains, 4 banks) at a time, kt-outer; bufs=2
                # lets mc+1's matmuls run while mc's banks evict
                for mc in range(4):
                    psm = psQK.tile([128, 4, 512], f32, tag="qk4")
                    for kt in range(8):
                        for sc in range(4):
                            nc.tensor.matmul(
                                psm[:, sc, :],
                                wqk_sb[:, kt, mc * 128 : (mc + 1) * 128],
                                embT_sb[:, kt, sc * 512 : (sc + 1) * 512],
                                start=(kt == 0),
                                stop=(kt == 7),
                            )
                    if mc < 2:  # Q: evict to q2 halves
                        he, ho = 2 * mc, 2 * mc + 1
                        for sc in range(4):
                            nc.scalar.activation(
                                out=q2_sb[0:64, he, sc * 512 : (sc + 1) * 512],
                                in_=psm[0:64, sc, :],
                                func=AF.Identity,
                                bias=bqk_sb[0:64, mc : mc + 1],
                                scale=1.0,
                            )
                            nc.scalar.activation(
                                out=q2_sb[64:128, ho, sc * 512 : (sc + 1) * 512],
                                in_=psm[64:128, sc, :],
                                func=AF.Identity,
                                bias=bqk_sb[64:128, mc : mc + 1],
                                scale=1.0,
                            )
                        # duplicate Q across halves (SBUF->SBUF DMA, chunked
                        # so the copies spread across DMA engines)
                        for h in (he, ho):
                            for c in range(4):
                                cs = slice(c * 512, (c + 1) * 512)
                                if h % 2 == 0:
                                    nc.sync.dma_start(
                                        out=q2_sb[64:128, h, cs], in_=q2_sb[0:64, h, cs]
                                    )
                                else:
                                    nc.sync.dma_start(
                                        out=q2_sb[0:64, h, cs], in_=q2_sb[64:128, h, cs]
                                    )
                    else:  # K: stage then scatter into row-tiled layout
                        he, ho = 2 * (mc - 2), 2 * (mc - 2) + 1
                        for sc in range(4):
                            kstage = work.tile([128, 512], bf16, tag="kstage")
                            nc.scalar.activation(
                                out=kstage[:],
                                in_=psm[:, sc, :],
                                func=AF.Identity,
                                bias=bqk_sb[:, mc : mc + 1],
                                scale=1.0,
                            )
                            # kstage[0:64] = K_he keys (kb 4sc..+3); [64:128] = K_ho
                            ks = kstage.rearrange("p (r m) -> p r m", r=4)
                            js = slice(2 * sc, 2 * sc + 2)
                            # even kbs (r=0,2) -> partitions 0:64; odd -> 64:128
                            nc.sync.dma_start(out=k_sb[0:64, he, js, :], in_=ks[0:64, 0::2, :])
                            nc.sync.dma_start(out=k_sb[64:128, he, js, :], in_=ks[0:64, 1::2, :])
                            nc.sync.dma_start(out=k_sb[0:64, ho, js, :], in_=ks[64:128, 0::2, :])
                            nc.sync.dma_start(out=k_sb[64:128, ho, js, :], in_=ks[64:128, 1::2, :])

            # V: [s, (h, a+1)] bf16, ones column LAST per head (sumexp row trick)
            v_sb = persist.tile([128, 16, HL, 1 + A], bf16)

            # xT: [(a), head pair, s] — heads stacked two per 128 partitions
            xT_sb = persist.tile([128, 2, S], bf16)

            with (
                tc.tile_pool(name="psA", bufs=2, space="PSUM") as psA,
                tc.tile_pool(name="psS", bufs=2, space="PSUM") as psS,
                tc.tile_pool(name="psB", bufs=2, space="PSUM") as psB,
            ):
                nc.vector.memset(v_sb, 1.0)

                def emit_v():
                    for st in range(16):
                        ps = psA.tile([128, HL * A], f32, tag="aux")
                        for kt in range(8):
                            nc.tensor.matmul(
                                ps[:],
                                embT_sb[:, kt, st * 128 : (st + 1) * 128],
                                wv_sb[:, kt, :],
                                start=(kt == 0),
                                stop=False,
                            )
                        nc.tensor.matmul(ps[:], ones_b[:, :], bv_sb[:, :], start=False, stop=True)
                        nc.vector.tensor_copy(
                            out=v_sb[:, st, :, 0:A],
                            in_=ps.rearrange("p (h a) -> p h a", h=HL),
                        )

                # ---------- attention, software-pipelined two units deep ----------
                def pv_mms(pu, kb0, kb1):
                    if pu["ps_x"] is None:
                        pu["ps_x"] = psB.tile([65, 512], f32, name="ps_x", tag="pvx")
                    for kb in range(kb0, kb1):
                        nc.tensor.matmul(
                            pu["ps_x"][:],
                            v_sb[:, kb, pu["h"], :],
                            pu["probs"][:, kb, :],
                            start=(kb == 0),
                            stop=(kb == 15),
                        )

                def finish_a(pu):
                    # stage sumexp + kick the DRAM broadcast-bounce early so
                    # the round trip never stalls the DVE FIFO (custom-DVE
                    # recip mislowers at base partition 64, hence the bounce
                    # to partitions 0-63)
                    pps_x = pu["ps_x"]
                    sexp = work.tile([65, 512], f32, tag="sexp")
                    nc.vector.tensor_copy(out=sexp[64:65, :], in_=pps_x[64:65, :])
                    se_d = dram.tile([1, 512], f32, name="sed", tag="sed", bufs=2)
                    nc.sync.dma_start(out=se_d[:, :], in_=sexp[64:65, :])
                    rb_sb = work.tile([64, 512], f32, tag="rbsb")
                    src = se_d[:, :]
                    bc = bass.AP(tensor=src.tensor, offset=src.offset, ap=[[0, 64], src.ap[-1]])
                    nc.sync.dma_start(out=rb_sb[:], in_=bc)
                    pu["rb"] = rb_sb

                def normalize_evict(pu):
                    pq, ph, pps_x = pu["q"], pu["h"], pu["ps_x"]
                    qo = pq * 512
                    recip = work.tile([64, 512], f32, tag="recip")
                    nc.vector.reciprocal_approx_fast(recip[:, :], pu["rb"][:, :])
                    if debug and pq == 0 and ph == 0:
                        dbg_psx_sb = work.tile([65, 512], f32, tag="dbgpsx", bufs=1)
                        nc.vector.tensor_copy(out=dbg_psx_sb[:], in_=pps_x[:, :])
                        nc.sync.dma_start(out=dbg_psx_d[:, :], in_=dbg_psx_sb[:])
                        nc.sync.dma_start(out=dbg_rcp_d[:, :], in_=recip[0:1, :])
                        nc.sync.dma_start(out=dbg_rb_d[:, :], in_=recip[:, :])
                    if ph % 2 == 0:
                        nc.vector.tensor_tensor(
                            xT_sb[0:64, ph // 2, qo : qo + 512],
                            pps_x[0:64, :],
                            recip[:, :],
                            OP.mult,
                        )
                    else:
                        xodd = work.tile([64, 512], bf16, tag="xodd")
                        nc.vector.tensor_tensor(
                            xodd[:], pps_x[0:64, :], recip[:, :], OP.mult
                        )
                        nc.sync.dma_start(
                            out=xT_sb[64:128, ph // 2, qo : qo + 512], in_=xodd[:]
                        )

                tails = {}

                def quarter_rs(q):
                    qo = q * 512
                    split = q == 3  # split the last RS by d-halves to shorten the tail
                    dcs = [[0], [1]] if split else [[0, 1]]
                    outs = []
                    for di, dgrp in enumerate(dcs):
                        dw = 512 * len(dgrp)
                        ar_in = dram.tile(
                            [QB, dw], bf16, name=f"arin{q}_{di}", tag=f"arin{q}_{di}"
                        )
                        ar_out = dram.tile(
                            [128, dw], bf16, name=f"arout{q}_{di}", tag=f"arout{q}_{di}"
                        )
                        for dc in dgrp:
                            for qc in range(4):
                                ps_o = psA.tile([128, 512], f32, tag="aux")
                                for hp in range(2):
                                    nc.tensor.matmul(
                                        ps_o[:],
                                        xT_sb[:, hp, qo + qc * 128 : qo + (qc + 1) * 128],
                                        wout_sb[:, hp, dc * 512 : (dc + 1) * 512],
                                        start=(hp == 0),
                                        stop=(hp == 1),
                                    )
                                oe = work.tile([128, 512], bf16, tag="oevict", bufs=3)
                                if qc % 2 == 0:
                                    nc.vector.tensor_copy(out=oe[:], in_=ps_o[:])
                                else:
                                    nc.scalar.copy(out=oe[:], in_=ps_o[:])
                                c0 = (dc - dgrp[0]) * 512
                                nc.sync.dma_start(
                                    out=ar_in[qc * 128 : (qc + 1) * 128, c0 : c0 + 512],
                                    in_=oe[:],
                                )
                        nc.gpsimd.collective_compute(
                            "ReduceScatter",
                            OP.add,
                            replica_groups=GROUPS,
                            ins=[ar_in[:, :].opt()],
                            outs=[ar_out[:, :].opt()],
                        )
                        outs.append((dgrp, ar_out))
                    tails[q] = outs

                ers = {}

                def quarter_ln(q):
                    rsl = slice(q * 128, (q + 1) * 128)
                    y = work.tile([128, D], f32, tag="y", bufs=1)
                    er = ers.pop(q)
                    stats = work.tile([128, 2, nc.vector.BN_STATS_DIM], f32, tag="stats")
                    rsb = work.tile([128, D], bf16, tag="rsb", bufs=1)
                    for dgrp, ar_out in tails.pop(q):
                        c0 = dgrp[0] * 512
                        dw = 512 * len(dgrp)
                        for ch in range(0, dw, 256):
                            nc.sync.dma_start(
                                out=rsb[:, c0 + ch : c0 + ch + 256],
                                in_=ar_out[:, ch : ch + 256],
                            )
                        for dc in dgrp:
                            hsl = slice(dc * 512, (dc + 1) * 512)
                            nc.vector.tensor_tensor(y[:, hsl], er[:, hsl], rsb[:, hsl], OP.add)
                            nc.vector.bn_stats(out=stats[:, dc, :], in_=y[:, hsl])
                    mv = work.tile([128, nc.vector.BN_AGGR_DIM], f32, tag="mv")
                    nc.vector.bn_aggr(out=mv[:], in_=stats[:])
                    rstd = work.tile([128, 1], f32, tag="rstd")
                    nc.scalar.activation(
                        out=rstd[:], in_=mv[:, 1:2], func=AF.Sqrt, bias=eps_sb[:], scale=1.0
                    )
                    nc.vector.reciprocal(rstd[:], rstd[:])
                    yn = work.tile([128, D], bf16, tag="yn", bufs=1)
                    nc.vector.tensor_scalar(
                        yn[:], y[:], mv[:, 0:1], rstd[:], OP.subtract, OP.mult
                    )
                    o = work.tile([128, D], bf16, tag="obf", bufs=1)
                    nc.vector.tensor_tensor(o[:], yn[:], gammabc[:], OP.mult)
                    nc.vector.tensor_tensor(o[:], o[:], betabc[:], OP.add)
                    for c in range(4):
                        nc.sync.dma_start(
                            out=out_d[rsl, c * 256 : (c + 1) * 256],
                            in_=o[:, c * 256 : (c + 1) * 256],
                        )

                def finish(pu):
                    normalize_evict(pu)
                    if pu["h"] == 3:
                        quarter_rs(pu["q"])

                # per-kb mask DMAs spread the 2MB load across all 16 DMA
                # engines (a single strided DMA serializes on one engine)
                def load_mask(q):
                    mq = work.tile([128, 16, 512], bf16, name="mq", tag="maskq")
                    for kb in range(16):
                        nc.sync.dma_start(
                            out=mq[:, kb, :],
                            in_=maskT_d[kb * 128 : (kb + 1) * 128, q * 512 : (q + 1) * 512],
                        )
                    return mq

                def load_er(q):
                    er = erp.tile([128, D], f32, tag="er")
                    for c in range(4):
                        nc.sync.dma_start(
                            out=er[:, c * 256 : (c + 1) * 256],
                            in_=embres_d[q * 128 : (q + 1) * 128, c * 256 : (c + 1) * 256],
                        )
                    ers[q] = er

                units = []
                mqs = {0: load_mask(0)}
                load_er(0)
                load_er(1)
                for quarter in range(4):
                    qoff = quarter * 512
                    for h in range(4):
                        if h == 0 and quarter + 1 < 4:
                            mqs[quarter + 1] = load_mask(quarter + 1)
                        # er(q) reuses er(q-2)'s buffer (bufs=2): load only
                        # after quarter_ln(q-2) — the buffer's reader — has
                        # been emitted (it runs at (q, h1, j5)).
                        if h == 2 and quarter >= 2:
                            load_er(quarter)
                        mq = mqs[quarter]
                        probs = probsp.tile([128, 16, 512], bf16, tag="probs")
                        unit = {"q": quarter, "h": h, "probs": probs, "ps_x": None}
                        for j in range(8):  # kb pairs
                            ps_s = psS.tile([128, 2, 512], f32, tag="score")
                            nc.tensor.matmul(
                                ps_s[:, 0, :],
                                k_sb[0:64, h, j, :],
                                q2_sb[0:64, h, qoff : qoff + 512],
                                start=True,
                                stop=True,
                                tile_position=(0, 0),
                            )
                            nc.tensor.matmul(
                                ps_s[:, 1, :],
                                k_sb[64:128, h, j, :],
                                q2_sb[64:128, h, qoff : qoff + 512],
                                start=True,
                                stop=True,
                                tile_position=(64, 0),
                            )
                            if units:
                                pv_mms(units[-1], 2 * j, 2 * j + 2)
                            if j < N_SCH:
                                # fused softmax+mask: bf16 bits of exp(s/8)
                                # built as (i16)(s*(184.665/8) + maskc2);
                                # masked lanes saturate to 0x8000 = -0.0
                                nc.vector.scalar_tensor_tensor(
                                    out=probs[:, 2 * j : 2 * j + 2, :].bitcast(i16dt),
                                    in0=ps_s[:, :, :],
                                    scalar=23.08312606,
                                    in1=mq[:, 2 * j : 2 * j + 2, :],
                                    op0=OP.mult,
                                    op1=OP.add,
                                )
                            else:
                                nc.scalar.activation(
                                    out=probs[:, 2 * j : 2 * j + 2, :],
                                    in_=ps_s[:, :, :],
                                    func=AF.Exp,
                                    scale=0.125,
                                )
                            if j == 3:  # mask for the ACT-path kbs of jobs 2,3
                                nc.vector.tensor_tensor(
                                    probs[:, 2 * N_SCH : 8, :],
                                    probs[:, 2 * N_SCH : 8, :],
                                    mq[:, 2 * N_SCH : 8, :],
                                    OP.mult,
                                )
                            elif j == 7:
                                nc.vector.tensor_tensor(
                                    probs[:, 8:16, :],
                                    probs[:, 8:16, :],
                                    mq[:, 8:16, :],
                                    OP.mult,
                                )
                            if j == 1 and len(units) >= 2:
                                finish_a(units[-2])
                            if j == 5 and len(units) >= 2:
                                finish(units[-2])
                            if j == 5 and quarter >= 2 and h == 1:
                                quarter_ln(quarter - 2)
                            if j == 5 and quarter == 3 and h == 3:
                                quarter_ln(2)
                        units.append(unit)
                        if quarter == 0 and h == 0:
                            emit_v()
                        if debug and quarter == 0 and h == 1:
                            nc.sync.dma_start(out=dbg_p0_d[:, :, :], in_=units[0]["probs"][:, :, :])
                        if debug and quarter == 0 and h == 3:
                            nc.sync.dma_start(out=dbg_mq_d[:, :, :], in_=mqs[0][:, :, :])
                finish_a(units[-2])
                finish(units[-2])
                pv_mms(units[-1], 0, 16)
                finish_a(units[-1])
                finish(units[-1])
                quarter_ln(3)
                if debug:
                    nc.sync.dma_start(out=dbg_q2_d[:, :, :], in_=q2_sb[:, :, :])
                    nc.sync.dma_start(out=dbg_k_d[:, :, :, :], in_=k_sb[:, :, :, :])
                    nc.sync.dma_start(out=dbg_v_d[:, :, :, :], in_=v_sb[:, :, :, :])
                    nc.sync.dma_start(out=dbg_xT_d[:, :, :], in_=xT_sb[:, :, :])

    nc.compile()
    return nc


def _prep_inputs(embeddings, attention_mask, W_qkv, b_qkv, W_out, b_out, ln_gamma, ln_beta):
    emb = np.asarray(embeddings, dtype=np.float32)
    mask = np.asarray(attention_mask)
    W_qkv = np.asarray(W_qkv, dtype=np.float32)
    b_qkv = np.asarray(b_qkv, dtype=np.float32)
    W_out = np.asarray(W_out, dtype=np.float32)
    b_out = np.asarray(b_out, dtype=np.float32)
    gamma = np.asarray(ln_gamma, dtype=np.float32).reshape(1, D).astype(ml_dtypes.bfloat16)
    beta = np.asarray(ln_beta, dtype=np.float32).reshape(1, D).astype(ml_dtypes.bfloat16)

    in_maps = []
    for c in range(NCORES):
        b = c // G
        g = c % G
        hs = g * HL * A  # 256g
        embT = np.ascontiguousarray(emb[b].T).astype(ml_dtypes.bfloat16)
        # keys in the first N_SCH kb-pairs carry the Schraudolph additive
        # constants (16256 live / -65536 masked); the rest carry 1/0.
        maskT = np.ascontiguousarray(mask[b].T).astype(np.float32)
        ksch = 256 * N_SCH
        maskT[:ksch] = np.where(maskT[:ksch] > 0, 16256.0, -65536.0)
        maskT = maskT.astype(ml_dtypes.bfloat16)
        wqk = np.ascontiguousarray(
            np.concatenate([W_qkv[:, hs : hs + 256], W_qkv[:, D + hs : D + hs + 256]], axis=1)
        ).astype(ml_dtypes.bfloat16)
        wv = np.ascontiguousarray(W_qkv[:, 2 * D + hs : 2 * D + hs + 256]).astype(
            ml_dtypes.bfloat16
        )
        bqk = np.concatenate([b_qkv[hs : hs + 256], b_qkv[D + hs : D + hs + 256]])
        bqk = np.ascontiguousarray(bqk.reshape(4, 128).T)
        bv = np.ascontiguousarray(
            b_qkv[2 * D + hs : 2 * D + hs + 256].reshape(1, 256)
        ).astype(ml_dtypes.bfloat16)
        wout = np.ascontiguousarray(
            W_out[hs : hs + 256, :].reshape(2, 128, D).transpose(1, 0, 2)
        ).astype(ml_dtypes.bfloat16)
        embres = np.concatenate(
            [emb[b, 512 * q + 128 * g : 512 * q + 128 * g + 128, :] for q in range(4)],
            axis=0,
        ) + b_out.reshape(1, D)
        in_maps.append(
            {
                "embT": embT,
                "embres": np.ascontiguousarray(embres.astype(np.float32)),
                "maskT": maskT,
                "wqk": wqk,
                "wv": wv,
                "bqk": bqk,
                "bv": bv,
                "onesb": np.ones((1, 128), dtype=ml_dtypes.bfloat16),
                "wout": wout,
                "gamma": gamma,
                "beta": beta,
            }
        )
    return in_maps


def _run(inputs, trace=False, **kw):
    if "nc" not in _CACHE:
        _CACHE["nc"] = _build()
    nc = _CACHE["nc"]
    in_maps = _prep_inputs(**inputs)
    res = run_bass_kernel_spmd(nc, in_maps, list(range(NCORES)), trace=trace, **kw)
    out = np.empty((B, S, D), dtype=np.float32)
    for c in range(NCORES):
        b, g = c // G, c % G
        for q in range(4):
            out[b, 512 * q + 128 * g : 512 * q + 128 * g + 128, :] = (
                res.results[c]["out"][128 * q : 128 * (q + 1), :].astype(np.float32)
            )
    return out, res


def kernel(**inputs):
    out, _ = _run(inputs, trace=False)
    return out


# revision 33
# speedup vs baseline: 1.1602x; 1.1602x over previous
"""Multi-head self-attention block (B=2, S=2048, D=1024, H=16) on 8 TRN2 cores.

Sharding: 2-way data-parallel over batch x 4-way tensor-parallel over heads.
Core c handles batch b=c//4 with group rank g=c%4 (heads 4g..4g+4). The
out-projection partials are combined with one bf16 ReduceScatter per
q-quarter over the 4-core batch group, so core g owns output rows
[512q + 128g, 512q + 128(g+1)) for q in 0..4 — collectives overlap the
remaining attention quarters instead of forming a serial tail.

Score matmuls are row-tiled: the contraction is only A=64, so two
concurrent 64-row PE tiles (tile_position (0,0)/(64,0)) each compute a
full 128-key x 512-query score block — the array runs at ~2x the naive
block-diagonal packing. K is stored [64 partitions][head][kb-pair][key]
with even key-blocks on partitions 0-63 and odd on 64-127.

QKV runs kt-outer across 8 PSUM banks (scoped pool) so the first matmul
only waits for the first 128-row chunk of W_qkv/embT instead of the
full 5MB load.

Self-contained: hardcodes all shapes; builds the Bass program once.
"""

import os
import sys

sys.path.insert(0, "/opt/trn_rl_repo")

import numpy as np
import ml_dtypes

import concourse.bass as bass
import concourse.tile as tile
from concourse import bacc, mybir
from concourse.bass_utils import run_bass_kernel_spmd

B, S, D, H = 2, 2048, 1024, 16
A = D // H  # 64
NCORES = 8
G = 4  # cores per batch group
HL = H // G  # local heads per core = 4
M_QK = 2 * HL * A  # 512 rows of Q_T+K_T per core
QB = S // G  # 512
EPS = 1e-3
GROUPS = [[0, 1, 2, 3], [4, 5, 6, 7]]

f32 = mybir.dt.float32
f32r = mybir.dt.float32r
bf16 = mybir.dt.bfloat16
i16dt = mybir.dt.int16

AF = mybir.ActivationFunctionType
OP = mybir.AluOpType

# First N_SCH kb-pairs of every unit use the fused DVE Schraudolph
# softmax (mask rows hold 16256 / -65536); the rest use ScalarE exp with
# a 1/0 mask multiply. Balances softmax work across DVE and ScalarE.
N_SCH = 2

_CACHE = {}


def _build():
    nc = bacc.Bacc("TRN2", target_bir_lowering=False, debug=False, num_devices=NCORES)

    # ---- I/O ----
    embT_d = nc.dram_tensor("embT", [D, S], bf16, kind="ExternalInput")
    embres_d = nc.dram_tensor("embres", [QB, D], bf16, kind="ExternalInput")
    maskT_d = nc.dram_tensor("maskT", [S, S], bf16, kind="ExternalInput")
    wqk_d = nc.dram_tensor("wqk", [D, M_QK], bf16, kind="ExternalInput")
    wv_d = nc.dram_tensor("wv", [D, HL * A], bf16, kind="ExternalInput")
    bqk_d = nc.dram_tensor("bqk", [128, 4], f32, kind="ExternalInput")
    bv_d = nc.dram_tensor("bv", [1, HL * A], bf16, kind="ExternalInput")
    onesb_d = nc.dram_tensor("onesb", [1, 128], bf16, kind="ExternalInput")
    wout_d = nc.dram_tensor("wout", [128, 2, D], bf16, kind="ExternalInput")
    gamma_d = nc.dram_tensor("gamma", [1, D], bf16, kind="ExternalInput")
    beta_d = nc.dram_tensor("beta", [1, D], bf16, kind="ExternalInput")
    out_d = nc.dram_tensor("out", [QB, D], bf16, kind="ExternalOutput")

    debug = bool(os.environ.get("KERNEL_DEBUG_DUMP"))
    if debug:
        dbg_q2_d = nc.dram_tensor("dbg_q2", [128, HL, S], bf16, kind="ExternalOutput")
        dbg_k_d = nc.dram_tensor("dbg_k", [128, HL, 8, 128], bf16, kind="ExternalOutput")
        dbg_v_d = nc.dram_tensor("dbg_v", [128, 16, HL, 1 + A], bf16, kind="ExternalOutput")
        dbg_mq_d = nc.dram_tensor("dbg_mq", [128, 16, 512], bf16, kind="ExternalOutput")
        dbg_p0_d = nc.dram_tensor("dbg_p0", [128, 16, 512], bf16, kind="ExternalOutput")
        dbg_xT_d = nc.dram_tensor("dbg_xT", [128, 2, S], bf16, kind="ExternalOutput")
        dbg_psx_d = nc.dram_tensor("dbg_psx", [65, 512], f32, kind="ExternalOutput")
        dbg_rcp_d = nc.dram_tensor("dbg_rcp", [1, 512], f32, kind="ExternalOutput")
        dbg_rb_d = nc.dram_tensor("dbg_rb", [64, 512], f32, kind="ExternalOutput")

    with tile.TileContext(nc) as tc:
        with (
            tc.tile_pool(name="big", bufs=1) as big,
            tc.tile_pool(name="persist", bufs=1) as persist,
            tc.tile_pool(name="probs", bufs=2) as probsp,
            tc.tile_pool(name="work", bufs=2) as work,
            tc.tile_pool(name="erp", bufs=4) as erp,
            tc.tile_pool(name="dram", bufs=1, space="DRAM") as dram,
        ):
            # ---------- interleaved W/embT chunk loads: QKV gates on chunk 0 ----------
            embT_sb = big.tile([128, 8, S], bf16, tag="bigslot")
            wqk_sb = persist.tile([128, 8, M_QK], bf16)
            for kt in range(8):
                nc.sync.dma_start(out=wqk_sb[:, kt, :], in_=wqk_d[kt * 128 : (kt + 1) * 128, :])
                nc.sync.dma_start(out=embT_sb[:, kt, :], in_=embT_d[kt * 128 : (kt + 1) * 128, :])
            bqk_sb = persist.tile([128, 4], f32)
            nc.sync.dma_start(out=bqk_sb, in_=bqk_d[:, :])

            wv_sb = persist.tile([128, 8, HL * A], bf16)
            for kt in range(8):
                nc.sync.dma_start(out=wv_sb[:, kt, :], in_=wv_d[kt * 128 : (kt + 1) * 128, :])
            bv_sb = persist.tile([1, HL * A], bf16)
            nc.sync.dma_start(out=bv_sb, in_=bv_d[:, :])
            ones_b = persist.tile([1, 128], bf16)
            nc.sync.dma_start(out=ones_b, in_=onesb_d[:, :])
            wout_sb = persist.tile([128, 2, D], bf16)
            nc.sync.dma_start(out=wout_sb, in_=wout_d[:, :, :])
            eps_sb = persist.tile([128, 1], f32)
            nc.vector.memset(eps_sb, EPS)
            gammabc = persist.tile([128, D], bf16)
            betabc = persist.tile([128, D], bf16)
            for t, dr in ((gammabc, gamma_d), (betabc, beta_d)):
                src = dr[:, :]
                bc = bass.AP(tensor=src.tensor, offset=src.offset, ap=[[0, 128], src.ap[1]])
                nc.sync.dma_start(out=t[:], in_=bc)

            # ---------- QKV projection (kt-outer over 8 PSUM banks) ----------
            # Q duplicated on both partition halves: q2[p, h, s], p<64 and
            # p>=64 both hold Q_h[p % 64, s].
            q2_sb = persist.tile([128, HL, S], bf16)
            # K row-tiled: k_sb[0:64, h, j, m] = K_h[a, keys of kb 2j],
            #              k_sb[64:128, h, j, m] = K_h[a, keys of kb 2j+1].
            k_sb = persist.tile([128, HL, 8, 128], bf16)

            with tc.tile_pool(name="psQK", bufs=2, space="PSUM") as psQK:
                # one mc (4 sc-chains, 4 banks) at a time, kt-outer; bufs=2
                # lets mc+1's matmuls run while mc's banks evict
                for mc in range(4):
                    psm = psQK.tile([128, 4, 512], f32, tag="qk4")
                    for kt in range(8):
                        for sc in range(4):
                            nc.tensor.matmul(
                                psm[:, sc, :],
                                wqk_sb[:, kt, mc * 128 : (mc + 1) * 128],
                                embT_sb[:, kt, sc * 512 : (sc + 1) * 512],
                                start=(kt == 0),
                                stop=(kt == 7),
                            )
                    if mc < 2:  # Q: evict to q2 halves
                        he, ho = 2 * mc, 2 * mc + 1
                        for sc in range(4):
                            nc.scalar.activation(
                                out=q2_sb[0:64, he, sc * 512 : (sc + 1) * 512],
                                in_=psm[0:64, sc, :],
                                func=AF.Identity,
                                bias=bqk_sb[0:64, mc : mc + 1],
                                scale=1.0,
                            )
                            nc.scalar.activation(
                                out=q2_sb[64:128, ho, sc * 512 : (sc + 1) * 512],
                                in_=psm[64:128, sc, :],
                                func=AF.Identity,
                                bias=bqk_sb[64:128, mc : mc + 1],
                                scale=1.0,
                            )
                        # duplicate Q across halves (SBUF->SBUF DMA, chunked
                        # so the copies spread across DMA engines)
                        for h in (he, ho):
                            for c in range(4):
                                cs = slice(c * 512, (c + 1) * 512)
                                if h % 2 == 0:
                                    nc.sync.dma_start(
                                        out=q2_sb[64:128, h, cs], in_=q2_sb[0:64, h, cs]
                                    )
                                else:
                                    nc.sync.dma_start(
                                        out=q2_sb[0:64, h, cs], in_=q2_sb[64:128, h, cs]
                                    )
                    else:  # K: stage then scatter into row-tiled layout
                        he, ho = 2 * (mc - 2), 2 * (mc - 2) + 1
                        for sc in range(4):
                            kstage = work.tile([128, 512], bf16, tag="kstage")
                            nc.scalar.activation(
                                out=kstage[:],
                                in_=psm[:, sc, :],
                                func=AF.Identity,
                                bias=bqk_sb[:, mc : mc + 1],
                                scale=1.0,
                            )
                            # kstage[0:64] = K_he keys (kb 4sc..+3); [64:128] = K_ho
                            ks = kstage.rearrange("p (r m) -> p r m", r=4)
                            js = slice(2 * sc, 2 * sc + 2)
                            # even kbs (r=0,2) -> partitions 0:64; odd -> 64:128
                            nc.sync.dma_start(out=k_sb[0:64, he, js, :], in_=ks[0:64, 0::2, :])
                            nc.sync.dma_start(out=k_sb[64:128, he, js, :], in_=ks[0:64, 1::2, :])
                            nc.sync.dma_start(out=k_sb[0:64, ho, js, :], in_=ks[64:128, 0::2, :])
                            nc.sync.dma_start(out=k_sb[64:128, ho, js, :], in_=ks[64:128, 1::2, :])

            # V: [s, (h, a+1)] bf16, ones column LAST per head (sumexp row trick)
            v_sb = persist.tile([128, 16, HL, 1 + A], bf16)

            # xT: [(a), head pair, s] — heads stacked two per 128 partitions
            xT_sb = persist.tile([128, 2, S], bf16)

            with (
                tc.tile_pool(name="psA", bufs=2, space="PSUM") as psA,
                tc.tile_pool(name="psS", bufs=2, space="PSUM") as psS,
                tc.tile_pool(name="psB", bufs=2, space="PSUM") as psB,
            ):
                nc.vector.memset(v_sb, 1.0)

                def emit_v():
                    for st in range(16):
                        ps = psA.tile([128, HL * A], f32, tag="aux")
                        for kt in range(8):
                            nc.tensor.matmul(
                                ps[:],
                                embT_sb[:, kt, st * 128 : (st + 1) * 128],
                                wv_sb[:, kt, :],
                                start=(kt == 0),
                                stop=False,
                            )
                        nc.tensor.matmul(ps[:], ones_b[:, :], bv_sb[:, :], start=False, stop=True)
                        nc.vector.tensor_copy(
                            out=v_sb[:, st, :, 0:A],
                            in_=ps.rearrange("p (h a) -> p h a", h=HL),
                        )

                # ---------- attention, software-pipelined two units deep ----------
                def pv_mms(pu, kb0, kb1):
                    if pu["ps_x"] is None:
                        pu["ps_x"] = psB.tile([65, 512], f32, name="ps_x", tag="pvx")
                    for kb in range(kb0, kb1):
                        nc.tensor.matmul(
                            pu["ps_x"][:],
                            v_sb[:, kb, pu["h"], :],
                            pu["probs"][:, kb, :],
                            start=(kb == 0),
                            stop=(kb == 15),
                        )

                def finish_a(pu):
                    # stage sumexp + kick the DRAM broadcast-bounce early so
                    # the round trip never stalls the DVE FIFO (custom-DVE
                    # recip mislowers at base partition 64, hence the bounce
                    # to partitions 0-63)
                    pps_x = pu["ps_x"]
                    sexp = work.tile([65, 512], f32, tag="sexp")
                    nc.vector.tensor_copy(out=sexp[64:65, :], in_=pps_x[64:65, :])
                    se_d = dram.tile([1, 512], f32, name="sed", tag="sed", bufs=2)
                    nc.sync.dma_start(out=se_d[:, :], in_=sexp[64:65, :])
                    rb_sb = work.tile([64, 512], f32, tag="rbsb")
                    src = se_d[:, :]
                    bc = bass.AP(tensor=src.tensor, offset=src.offset, ap=[[0, 64], src.ap[-1]])
                    nc.sync.dma_start(out=rb_sb[:], in_=bc)
                    pu["rb"] = rb_sb

                def normalize_evict(pu):
                    pq, ph, pps_x = pu["q"], pu["h"], pu["ps_x"]
                    qo = pq * 512
                    recip = work.tile([64, 512], f32, tag="recip")
                    nc.vector.reciprocal_approx_fast(recip[:, :], pu["rb"][:, :])
                    if debug and pq == 0 and ph == 0:
                        dbg_psx_sb = work.tile([65, 512], f32, tag="dbgpsx", bufs=1)
                        nc.vector.tensor_copy(out=dbg_psx_sb[:], in_=pps_x[:, :])
                        nc.sync.dma_start(out=dbg_psx_d[:, :], in_=dbg_psx_sb[:])
                        nc.sync.dma_start(out=dbg_rcp_d[:, :], in_=recip[0:1, :])
                        nc.sync.dma_start(out=dbg_rb_d[:, :], in_=recip[:, :])
                    if ph % 2 == 0:
                        nc.vector.tensor_tensor(
                            xT_sb[0:64, ph // 2, qo : qo + 512],
                            pps_x[0:64, :],
                            recip[:, :],
                            OP.mult,
                        )
                    else:
                        xodd = work.tile([64, 512], bf16, tag="xodd")
                        nc.vector.tensor_tensor(
                            xodd[:], pps_x[0:64, :], recip[:, :], OP.mult
                        )
                        nc.sync.dma_start(
                            out=xT_sb[64:128, ph // 2, qo : qo + 512], in_=xodd[:]
                        )

                tails = {}

                def quarter_rs(q):
                    qo = q * 512
                    split = q == 3  # split the last RS by d-halves to shorten the tail
                    dcs = [[0], [1]] if split else [[0, 1]]
                    outs = []
                    for di, dgrp in enumerate(dcs):
                        dw = 512 * len(dgrp)
                        ar_in = dram.tile(
                            [QB, dw], bf16, name=f"arin{q}_{di}", tag=f"arin{q}_{di}"
                        )
                        ar_out = dram.tile(
                            [128, dw], bf16, name=f"arout{q}_{di}", tag=f"arout{q}_{di}"
                        )
                        for dc in dgrp:
                            for qc in range(4):
                                ps_o = psA.tile([128, 512], f32, tag="aux")
                                for hp in range(2):
                                    nc.tensor.matmul(
                                        ps_o[:],
                                        xT_sb[:, hp, qo + qc * 128 : qo + (qc + 1) * 128],
                                        wout_sb[:, hp, dc * 512 : (dc + 1) * 512],
                                        start=(hp == 0),
                                        stop=(hp == 1),
                                    )
                                oe = work.tile([128, 512], bf16, tag="oevict", bufs=3)
                                if qc % 2 == 0:
                                    nc.vector.tensor_copy(out=oe[:], in_=ps_o[:])
                                else:
                                    nc.scalar.copy(out=oe[:], in_=ps_o[:])
                                c0 = (dc - dgrp[0]) * 512
                                nc.sync.dma_start(
                                    out=ar_in[qc * 128 : (qc + 1) * 128, c0 : c0 + 512],
                                    in_=oe[:],
                                )
                        nc.gpsimd.collective_compute(
                            "ReduceScatter",
                            OP.add,
                            replica_groups=GROUPS,
                            ins=[ar_in[:, :].opt()],
                            outs=[ar_out[:, :].opt()],
                        )
                        outs.append((dgrp, ar_out))
                    tails[q] = outs

                ers = {}

                def quarter_ln(q):
                    rsl = slice(q * 128, (q + 1) * 128)
                    y = work.tile([128, D], f32, tag="y", bufs=2)
                    er = ers.pop(q)
                    stats = work.tile([128, 2, nc.vector.BN_STATS_DIM], f32, tag="stats")
                    rsb = work.tile([128, D], bf16, tag="rsb", bufs=2)
                    for dgrp, ar_out in tails.pop(q):
                        c0 = dgrp[0] * 512
                        dw = 512 * len(dgrp)
                        for ch in range(0, dw, 256):
                            nc.sync.dma_start(
                                out=rsb[:, c0 + ch : c0 + ch + 256],
                                in_=ar_out[:, ch : ch + 256],
                            )
                        for dc in dgrp:
                            hsl = slice(dc * 512, (dc + 1) * 512)
                            nc.vector.tensor_tensor(y[:, hsl], er[:, hsl], rsb[:, hsl], OP.add)
                            nc.vector.bn_stats(out=stats[:, dc, :], in_=y[:, hsl])
                    mv = work.tile([128, nc.vector.BN_AGGR_DIM], f32, tag="mv")
                    nc.vector.bn_aggr(out=mv[:], in_=stats[:])
                    rstd = work.tile([128, 1], f32, tag="rstd")
                    nc.scalar.activation(
                        out=rstd[:], in_=mv[:, 1:2], func=AF.Sqrt, bias=eps_sb[:], scale=1.0
                    )
                    nc.vector.reciprocal(rstd[:], rstd[:])
                    yn = work.tile([128, D], bf16, tag="yn", bufs=2)
                    nc.vector.tensor_scalar(
                        yn[:], y[:], mv[:, 0:1], rstd[:], OP.subtract, OP.mult
                    )
                    o = work.tile([128, D], bf16, tag="obf", bufs=2)
                    nc.vector.tensor_tensor(o[:], yn[:], gammabc[:], OP.mult)
                    nc.vector.tensor_tensor(o[:], o[:], betabc[:], OP.add)
                    for c in range(4):
                        nc.sync.dma_start(
                            out=out_d[rsl, c * 256 : (c + 1) * 256],
                            in_=o[:, c * 256 : (c + 1) * 256],
                        )

                def finish(pu):
                    normalize_evict(pu)
                    if pu["h"] == 3:
                        quarter_rs(pu["q"])

                # per-kb mask DMAs spread the 2MB load across all 16 DMA
                # engines (a single strided DMA serializes on one engine)
                def load_mask(q):
                    mq = work.tile([128, 16, 512], bf16, name="mq", tag="maskq")
                    for kb in range(16):
                        nc.sync.dma_start(
                            out=mq[:, kb, :],
                            in_=maskT_d[kb * 128 : (kb + 1) * 128, q * 512 : (q + 1) * 512],
                        )
                    return mq

                def load_er(q):
                    er = erp.tile([128, D], bf16, tag="er")
                    for c in range(4):
                        nc.sync.dma_start(
                            out=er[:, c * 256 : (c + 1) * 256],
                            in_=embres_d[q * 128 : (q + 1) * 128, c * 256 : (c + 1) * 256],
                        )
                    ers[q] = er

                units = []
                mqs = {0: load_mask(0)}
                for q in range(4):
                    load_er(q)
                for quarter in range(4):
                    qoff = quarter * 512
                    for h in range(4):
                        if h == 0 and quarter + 1 < 4:
                            mqs[quarter + 1] = load_mask(quarter + 1)
                        mq = mqs[quarter]
                        probs = probsp.tile([128, 16, 512], bf16, tag="probs")
                        unit = {"q": quarter, "h": h, "probs": probs, "ps_x": None}
                        for j in range(8):  # kb pairs
                            ps_s = psS.tile([128, 2, 512], f32, tag="score")
                            nc.tensor.matmul(
                                ps_s[:, 0, :],
                                k_sb[0:64, h, j, :],
                                q2_sb[0:64, h, qoff : qoff + 512],
                                start=True,
                                stop=True,
                                tile_position=(0, 0),
                            )
                            nc.tensor.matmul(
                                ps_s[:, 1, :],
                                k_sb[64:128, h, j, :],
                                q2_sb[64:128, h, qoff : qoff + 512],
                                start=True,
                                stop=True,
                                tile_position=(64, 0),
                            )
                            if units:
                                pv_mms(units[-1], 2 * j, 2 * j + 2)
                            if j < N_SCH:
                                # fused softmax+mask: bf16 bits of exp(s/8)
                                # built as (i16)(s*(184.665/8) + maskc2);
                                # masked lanes saturate to 0x8000 = -0.0
                                nc.vector.scalar_tensor_tensor(
                                    out=probs[:, 2 * j : 2 * j + 2, :].bitcast(i16dt),
                                    in0=ps_s[:, :, :],
                                    scalar=23.08312606,
                                    in1=mq[:, 2 * j : 2 * j + 2, :],
                                    op0=OP.mult,
                                    op1=OP.add,
                                )
                            else:
                                nc.scalar.activation(
                                    out=probs[:, 2 * j : 2 * j + 2, :],
                                    in_=ps_s[:, :, :],
                                    func=AF.Exp,
                                    scale=0.125,
                                )
                            if j == 3:  # mask for the ACT-path kbs of jobs 2,3
                                nc.vector.tensor_tensor(
                                    probs[:, 2 * N_SCH : 8, :],
                                    probs[:, 2 * N_SCH : 8, :],
                                    mq[:, 2 * N_SCH : 8, :],
                                    OP.mult,
                                )
                            elif j == 7:
                                nc.vector.tensor_tensor(
                                    probs[:, 8:16, :],
                                    probs[:, 8:16, :],
                                    mq[:, 8:16, :],
                                    OP.mult,
                                )
                            if j == 1 and len(units) >= 2:
                                finish_a(units[-2])
                            if j == 5 and len(units) >= 2:
                                finish(units[-2])
                        units.append(unit)
                        if quarter == 0 and h == 0:
                            emit_v()
                        if debug and quarter == 0 and h == 1:
                            nc.sync.dma_start(out=dbg_p0_d[:, :, :], in_=units[0]["probs"][:, :, :])
                        if debug and quarter == 0 and h == 3:
                            nc.sync.dma_start(out=dbg_mq_d[:, :, :], in_=mqs[0][:, :, :])
                # tail: issue RS(3) first, then run all LNs (collective-
                # coupled work happens only here, so a late-starting peer
                # can never stall the mid-stream engine FIFOs)
                finish_a(units[-2])
                finish(units[-2])
                pv_mms(units[-1], 0, 16)
                finish_a(units[-1])
                finish(units[-1])
                for q in range(4):
                    quarter_ln(q)
                if debug:
                    nc.sync.dma_start(out=dbg_q2_d[:, :, :], in_=q2_sb[:, :, :])
                    nc.sync.dma_start(out=dbg_k_d[:, :, :, :], in_=k_sb[:, :, :, :])
                    nc.sync.dma_start(out=dbg_v_d[:, :, :, :], in_=v_sb[:, :, :, :])
                    nc.sync.dma_start(out=dbg_xT_d[:, :, :], in_=xT_sb[:, :, :])

    nc.compile()
    return nc


def _prep_inputs(embeddings, attention_mask, W_qkv, b_qkv, W_out, b_out, ln_gamma, ln_beta):
    emb = np.asarray(embeddings, dtype=np.float32)
    mask = np.asarray(attention_mask)
    W_qkv = np.asarray(W_qkv, dtype=np.float32)
    b_qkv = np.asarray(b_qkv, dtype=np.float32)
    W_out = np.asarray(W_out, dtype=np.float32)
    b_out = np.asarray(b_out, dtype=np.float32)
    gamma = np.asarray(ln_gamma, dtype=np.float32).reshape(1, D).astype(ml_dtypes.bfloat16)
    beta = np.asarray(ln_beta, dtype=np.float32).reshape(1, D).astype(ml_dtypes.bfloat16)

    in_maps = []
    for c in range(NCORES):
        b = c // G
        g = c % G
        hs = g * HL * A  # 256g
        embT = np.ascontiguousarray(emb[b].T).astype(ml_dtypes.bfloat16)
        # keys in the first N_SCH kb-pairs carry the Schraudolph additive
        # constants (16256 live / -65536 masked); the rest carry 1/0.
        maskT = np.ascontiguousarray(mask[b].T).astype(np.float32)
        ksch = 256 * N_SCH
        maskT[:ksch] = np.where(maskT[:ksch] > 0, 16256.0, -65536.0)
        maskT = maskT.astype(ml_dtypes.bfloat16)
        wqk = np.ascontiguousarray(
            np.concatenate([W_qkv[:, hs : hs + 256], W_qkv[:, D + hs : D + hs + 256]], axis=1)
        ).astype(ml_dtypes.bfloat16)
        wv = np.ascontiguousarray(W_qkv[:, 2 * D + hs : 2 * D + hs + 256]).astype(
            ml_dtypes.bfloat16
        )
        bqk = np.concatenate([b_qkv[hs : hs + 256], b_qkv[D + hs : D + hs + 256]])
        bqk = np.ascontiguousarray(bqk.reshape(4, 128).T)
        bv = np.ascontiguousarray(
            b_qkv[2 * D + hs : 2 * D + hs + 256].reshape(1, 256)
        ).astype(ml_dtypes.bfloat16)
        wout = np.ascontiguousarray(
            W_out[hs : hs + 256, :].reshape(2, 128, D).transpose(1, 0, 2)
        ).astype(ml_dtypes.bfloat16)
        embres = np.concatenate(
            [emb[b, 512 * q + 128 * g : 512 * q + 128 * g + 128, :] for q in range(4)],
            axis=0,
        ) + b_out.reshape(1, D)
        in_maps.append(
            {
                "embT": embT,
                "embres": np.ascontiguousarray(embres.astype(ml_dtypes.bfloat16)),
                "maskT": maskT,
                "wqk": wqk,
                "wv": wv,
                "bqk": bqk,
                "bv": bv,
                "onesb": np.ones((1, 128), dtype=ml_dtypes.bfloat16),
                "wout": wout,
                "gamma": gamma,
                "beta": beta,
            }
        )
    return in_maps


def _run(inputs, trace=False, **kw):
    if "nc" not in _CACHE:
        _CACHE["nc"] = _build()
    nc = _CACHE["nc"]
    in_maps = _prep_inputs(**inputs)
    res = run_bass_kernel_spmd(nc, in_maps, list(range(NCORES)), trace=trace, **kw)
    out = np.empty((B, S, D), dtype=np.float32)
    for c in range(NCORES):
        b, g = c // G, c % G
        for q in range(4):
            out[b, 512 * q + 128 * g : 512 * q + 128 * g + 128, :] = (
                res.results[c]["out"][128 * q : 128 * (q + 1), :].astype(np.float32)
            )
    return out, res


def kernel(**inputs):
    out, _ = _run(inputs, trace=False)
    return out


# revision 42
# speedup vs baseline: 1.3500x; 1.1636x over previous
"""Multi-head self-attention block (B=2, S=2048, D=1024, H=16) on 8 TRN2 cores.

Sharding: 2-way data-parallel over batch x 4-way tensor-parallel over heads.
Core c handles batch b=c//4 with group rank g=c%4 (heads 4g..4g+4). The
out-projection partials are combined with one bf16 ReduceScatter per
q-quarter over the 4-core batch group, so core g owns output rows
[512q + 128g, 512q + 128(g+1)) for q in 0..4 — collectives overlap the
remaining attention quarters instead of forming a serial tail.

Score matmuls are row-tiled: the contraction is only A=64, so two
concurrent 64-row PE tiles (tile_position (0,0)/(64,0)) each compute a
full 128-key x 512-query score block — the array runs at ~2x the naive
block-diagonal packing. K is stored [64 partitions][head][kb-pair][key]
with even key-blocks on partitions 0-63 and odd on 64-127.

QKV runs kt-outer across 8 PSUM banks (scoped pool) so the first matmul
only waits for the first 128-row chunk of W_qkv/embT instead of the
full 5MB load.

Self-contained: hardcodes all shapes; builds the Bass program once.
"""

import os
import sys

sys.path.insert(0, "/opt/trn_rl_repo")

import numpy as np
import ml_dtypes

import concourse.bass as bass
import concourse.tile as tile
from concourse import bacc, mybir
from concourse.bass_utils import run_bass_kernel_spmd

B, S, D, H = 2, 2048, 1024, 16
A = D // H  # 64
NCORES = 8
G = 4  # cores per batch group
HL = H // G  # local heads per core = 4
M_QK = 2 * HL * A  # 512 rows of Q_T+K_T per core
QB = S // G  # 512
EPS = 1e-3
GROUPS = [[0, 1, 2, 3], [4, 5, 6, 7]]

f32 = mybir.dt.float32
f32r = mybir.dt.float32r
bf16 = mybir.dt.bfloat16
i16dt = mybir.dt.int16

AF = mybir.ActivationFunctionType
OP = mybir.AluOpType

# First N_SCH kb-pairs of every unit use the fused DVE Schraudolph
# softmax (mask rows hold 16256 / -65536); the rest use ScalarE exp with
# a 1/0 mask multiply. Balances softmax work across DVE and ScalarE.
N_SCH = 2

# QKV runs in fp8 (DoubleRow, virtual k=256). Weights are scaled by WS
# so small weights stay out of the e4m3 subnormal range; Q,K come out
# WS-scaled making scores WS^2-scaled (folded into the exp constants),
# and V comes out WS-scaled (cancelled by the WS-valued ones column in
# the sumexp row).
WS = 64.0
f8e4 = mybir.dt.float8e4

_CACHE = {}


def _build():
    nc = bacc.Bacc("TRN2", target_bir_lowering=False, debug=False, num_devices=NCORES)

    # ---- I/O ----
    # embT8/wqk8/wv8 are DoubleRow-interleaved fp8: [p, j, ktp, cols]
    # holds element [256*ktp + 128*j + p, col] of the [D, cols] original.
    embT_d = nc.dram_tensor("embT", [128, 8 * S], f8e4, kind="ExternalInput")
    embres_d = nc.dram_tensor("embres", [QB, D], bf16, kind="ExternalInput")
    maskT_d = nc.dram_tensor("maskT", [S, S], bf16, kind="ExternalInput")
    wqk_d = nc.dram_tensor("wqk", [128, 8 * M_QK], f8e4, kind="ExternalInput")
    wv_d = nc.dram_tensor("wv", [128, 8 * HL * A], f8e4, kind="ExternalInput")
    bqk_d = nc.dram_tensor("bqk", [128, 4], f32, kind="ExternalInput")
    bv_d = nc.dram_tensor("bv", [1, HL * A], bf16, kind="ExternalInput")
    onesb_d = nc.dram_tensor("onesb", [1, 128], bf16, kind="ExternalInput")
    wout_d = nc.dram_tensor("wout", [128, 2, D], bf16, kind="ExternalInput")
    gamma_d = nc.dram_tensor("gamma", [1, D], bf16, kind="ExternalInput")
    beta_d = nc.dram_tensor("beta", [1, D], bf16, kind="ExternalInput")
    out_d = nc.dram_tensor("out", [QB, D], bf16, kind="ExternalOutput")

    debug = bool(os.environ.get("KERNEL_DEBUG_DUMP"))
    if debug:
        dbg_q2_d = nc.dram_tensor("dbg_q2", [128, HL, S], bf16, kind="ExternalOutput")
        dbg_k_d = nc.dram_tensor("dbg_k", [128, HL, 8, 128], bf16, kind="ExternalOutput")
        dbg_v_d = nc.dram_tensor("dbg_v", [128, 16, HL, 1 + A], bf16, kind="ExternalOutput")
        dbg_mq_d = nc.dram_tensor("dbg_mq", [128, 16, 512], bf16, kind="ExternalOutput")
        dbg_p0_d = nc.dram_tensor("dbg_p0", [128, 16, 512], bf16, kind="ExternalOutput")
        dbg_xT_d = nc.dram_tensor("dbg_xT", [128, 2, S], bf16, kind="ExternalOutput")
        dbg_psx_d = nc.dram_tensor("dbg_psx", [65, 512], f32, kind="ExternalOutput")
        dbg_rcp_d = nc.dram_tensor("dbg_rcp", [1, 512], f32, kind="ExternalOutput")
        dbg_rb_d = nc.dram_tensor("dbg_rb", [64, 512], f32, kind="ExternalOutput")

    with tile.TileContext(nc) as tc:
        with (
            tc.tile_pool(name="big", bufs=1) as big,
            tc.tile_pool(name="persist", bufs=1) as persist,
            tc.tile_pool(name="probs", bufs=2) as probsp,
            tc.tile_pool(name="work", bufs=2) as work,
            tc.tile_pool(name="erp", bufs=4) as erp,
            tc.tile_pool(name="dram", bufs=1, space="DRAM") as dram,
        ):
            # ---------- interleaved W/embT chunk loads: QKV gates on chunk 0 ----------
            # [128, j, ktp, cols] fp8 DoubleRow layout; slot = j*4 + ktp
            embT_sb = big.tile([128, 2, 4, S], f8e4, tag="bigslot")
            wqk_sb = persist.tile([128, 2, 4, M_QK], f8e4)
            embT_v = embT_sb.rearrange("p j k s -> p (j k) s")
            wqk_v = wqk_sb.rearrange("p j k s -> p (j k) s")
            for slot in range(8):
                nc.sync.dma_start(
                    out=wqk_v[:, slot, :], in_=wqk_d[:, slot * M_QK : (slot + 1) * M_QK]
                )
                nc.sync.dma_start(
                    out=embT_v[:, slot, :], in_=embT_d[:, slot * S : (slot + 1) * S]
                )
            bqk_sb = persist.tile([128, 4], f32)
            nc.sync.dma_start(out=bqk_sb, in_=bqk_d[:, :])

            wv_sb = persist.tile([128, 2, 4, HL * A], f8e4)
            wv_v = wv_sb.rearrange("p j k s -> p (j k) s")
            for slot in range(8):
                nc.sync.dma_start(
                    out=wv_v[:, slot, :], in_=wv_d[:, slot * HL * A : (slot + 1) * HL * A]
                )
            bv_sb = persist.tile([1, HL * A], bf16)
            nc.sync.dma_start(out=bv_sb, in_=bv_d[:, :])
            ones_b = persist.tile([1, 128], bf16)
            nc.sync.dma_start(out=ones_b, in_=onesb_d[:, :])
            wout_sb = persist.tile([128, 2, D], bf16)
            nc.sync.dma_start(out=wout_sb, in_=wout_d[:, :, :])
            eps_sb = persist.tile([128, 1], f32)
            nc.vector.memset(eps_sb, EPS)
            gammabc = persist.tile([128, D], bf16)
            betabc = persist.tile([128, D], bf16)
            for t, dr in ((gammabc, gamma_d), (betabc, beta_d)):
                src = dr[:, :]
                bc = bass.AP(tensor=src.tensor, offset=src.offset, ap=[[0, 128], src.ap[1]])
                nc.sync.dma_start(out=t[:], in_=bc)

            # ---------- QKV projection (kt-outer over 8 PSUM banks) ----------
            # Q duplicated on both partition halves: q2[p, h, s], p<64 and
            # p>=64 both hold Q_h[p % 64, s].
            q2_sb = persist.tile([128, HL, S], bf16)
            # K row-tiled: k_sb[0:64, h, j, m] = K_h[a, keys of kb 2j],
            #              k_sb[64:128, h, j, m] = K_h[a, keys of kb 2j+1].
            k_sb = persist.tile([128, HL, 8, 128], bf16)

            with tc.tile_pool(name="psQK", bufs=2, space="PSUM") as psQK:
                # one mc (4 sc-chains, 4 banks) at a time, kt-outer; bufs=2
                # lets mc+1's matmuls run while mc's banks evict
                for mc in range(4):
                    psm = psQK.tile([128, 4, 512], f32, tag="qk4")
                    for ktp in range(4):
                        for sc in range(4):
                            nc.tensor.matmul(
                                psm[:, sc, :],
                                wqk_sb[:, :, ktp, mc * 128 : (mc + 1) * 128],
                                embT_sb[:, :, ktp, sc * 512 : (sc + 1) * 512],
                                start=(ktp == 0),
                                stop=(ktp == 3),
                                perf_mode=mybir.MatmulPerfMode.DoubleRow,
                            )
                    if mc < 2:  # Q: evict to q2 halves
                        he, ho = 2 * mc, 2 * mc + 1
                        for sc in range(4):
                            nc.scalar.activation(
                                out=q2_sb[0:64, he, sc * 512 : (sc + 1) * 512],
                                in_=psm[0:64, sc, :],
                                func=AF.Identity,
                                bias=bqk_sb[0:64, mc : mc + 1],
                                scale=1.0,
                            )
                            nc.scalar.activation(
                                out=q2_sb[64:128, ho, sc * 512 : (sc + 1) * 512],
                                in_=psm[64:128, sc, :],
                                func=AF.Identity,
                                bias=bqk_sb[64:128, mc : mc + 1],
                                scale=1.0,
                            )
                        # duplicate Q across halves (SBUF->SBUF DMA, chunked
                        # so the copies spread across DMA engines)
                        for h in (he, ho):
                            for c in range(4):
                                cs = slice(c * 512, (c + 1) * 512)
                                if h % 2 == 0:
                                    nc.sync.dma_start(
                                        out=q2_sb[64:128, h, cs], in_=q2_sb[0:64, h, cs]
                                    )
                                else:
                                    nc.sync.dma_start(
                                        out=q2_sb[0:64, h, cs], in_=q2_sb[64:128, h, cs]
                                    )
                    else:  # K: stage then scatter into row-tiled layout
                        he, ho = 2 * (mc - 2), 2 * (mc - 2) + 1
                        for sc in range(4):
                            kstage = work.tile([128, 512], bf16, tag="kstage", bufs=4)
                            nc.scalar.activation(
                                out=kstage[:],
                                in_=psm[:, sc, :],
                                func=AF.Identity,
                                bias=bqk_sb[:, mc : mc + 1],
                                scale=1.0,
                            )
                            # kstage[0:64] = K_he keys (kb 4sc..+3); [64:128] = K_ho
                            ks = kstage.rearrange("p (r m) -> p r m", r=4)
                            js = slice(2 * sc, 2 * sc + 2)
                            # even kbs (r=0,2) -> partitions 0:64; odd -> 64:128
                            nc.sync.dma_start(out=k_sb[0:64, he, js, :], in_=ks[0:64, 0::2, :])
                            nc.sync.dma_start(out=k_sb[64:128, he, js, :], in_=ks[0:64, 1::2, :])
                            nc.sync.dma_start(out=k_sb[0:64, ho, js, :], in_=ks[64:128, 0::2, :])
                            nc.sync.dma_start(out=k_sb[64:128, ho, js, :], in_=ks[64:128, 1::2, :])

            # V: [s, (h, a+1)] bf16, ones column LAST per head (sumexp row trick)
            v_sb = persist.tile([128, 16, HL, 1 + A], bf16)

            # xT: [(a), head pair, s] — heads stacked two per 128 partitions
            xT_sb = persist.tile([128, 2, S], bf16)

            with (
                tc.tile_pool(name="psA", bufs=2, space="PSUM") as psA,
                tc.tile_pool(name="psS", bufs=2, space="PSUM") as psS,
                tc.tile_pool(name="psB", bufs=2, space="PSUM") as psB,
            ):
                nc.vector.memset(v_sb, WS)

                def emit_v():
                    for st in range(16):
                        ps = psA.tile([128, HL * A], f32, tag="aux")
                        for ktp in range(4):
                            nc.tensor.matmul(
                                ps[:],
                                embT_sb[:, :, ktp, st * 128 : (st + 1) * 128],
                                wv_sb[:, :, ktp, :],
                                start=(ktp == 0),
                                stop=False,
                                perf_mode=mybir.MatmulPerfMode.DoubleRow,
                            )
                        nc.tensor.matmul(ps[:], ones_b[:, :], bv_sb[:, :], start=False, stop=True)
                        nc.vector.tensor_copy(
                            out=v_sb[:, st, :, 0:A],
                            in_=ps.rearrange("p (h a) -> p h a", h=HL),
                        )

                # ---------- attention, software-pipelined two units deep ----------
                def pv_mms(pu, kb0, kb1):
                    if pu["ps_x"] is None:
                        pu["ps_x"] = psB.tile([65, 512], f32, name="ps_x", tag="pvx")
                    for kb in range(kb0, kb1):
                        nc.tensor.matmul(
                            pu["ps_x"][:],
                            v_sb[:, kb, pu["h"], :],
                            pu["probs"][:, kb, :],
                            start=(kb == 0),
                            stop=(kb == 15),
                        )

                def finish_a(pu):
                    # stage sumexp + kick the DRAM broadcast-bounce early so
                    # the round trip never stalls the DVE FIFO (custom-DVE
                    # recip mislowers at base partition 64, hence the bounce
                    # to partitions 0-63)
                    pps_x = pu["ps_x"]
                    sexp = work.tile([65, 512], f32, tag="sexp")
                    nc.vector.tensor_copy(out=sexp[64:65, :], in_=pps_x[64:65, :])
                    se_d = dram.tile([1, 512], f32, name="sed", tag="sed", bufs=2)
                    nc.sync.dma_start(out=se_d[:, :], in_=sexp[64:65, :])
                    rb_sb = work.tile([64, 512], f32, tag="rbsb")
                    src = se_d[:, :]
                    bc = bass.AP(tensor=src.tensor, offset=src.offset, ap=[[0, 64], src.ap[-1]])
                    nc.sync.dma_start(out=rb_sb[:], in_=bc)
                    pu["rb"] = rb_sb

                def normalize_evict(pu):
                    pq, ph, pps_x = pu["q"], pu["h"], pu["ps_x"]
                    qo = pq * 512
                    recip = work.tile([64, 512], f32, tag="recip")
                    nc.vector.reciprocal_approx_fast(recip[:, :], pu["rb"][:, :])
                    if debug and pq == 0 and ph == 0:
                        dbg_psx_sb = work.tile([65, 512], f32, tag="dbgpsx", bufs=1)
                        nc.vector.tensor_copy(out=dbg_psx_sb[:], in_=pps_x[:, :])
                        nc.sync.dma_start(out=dbg_psx_d[:, :], in_=dbg_psx_sb[:])
                        nc.sync.dma_start(out=dbg_rcp_d[:, :], in_=recip[0:1, :])
                        nc.sync.dma_start(out=dbg_rb_d[:, :], in_=recip[:, :])
                    if ph % 2 == 0:
                        nc.vector.tensor_tensor(
                            xT_sb[0:64, ph // 2, qo : qo + 512],
                            pps_x[0:64, :],
                            recip[:, :],
                            OP.mult,
                        )
                    else:
                        xodd = work.tile([64, 512], bf16, tag="xodd")
                        nc.vector.tensor_tensor(
                            xodd[:], pps_x[0:64, :], recip[:, :], OP.mult
                        )
                        nc.sync.dma_start(
                            out=xT_sb[64:128, ph // 2, qo : qo + 512], in_=xodd[:]
                        )

                tails = {}

                def quarter_rs(q):
                    qo = q * 512
                    dcs = [[0, 1]]
                    outs = []
                    for di, dgrp in enumerate(dcs):
                        dw = 512 * len(dgrp)
                        ar_in = dram.tile(
                            [QB, dw], bf16, name=f"arin{q}_{di}", tag=f"arin{q}_{di}"
                        )
                        ar_out = dram.tile(
                            [128, dw], bf16, name=f"arout{q}_{di}", tag=f"arout{q}_{di}"
                        )
                        for dc in dgrp:
                            for qc in range(4):
                                ps_o = psA.tile([128, 512], f32, tag="aux")
                                for hp in range(2):
                                    nc.tensor.matmul(
                                        ps_o[:],
                                        xT_sb[:, hp, qo + qc * 128 : qo + (qc + 1) * 128],
                                        wout_sb[:, hp, dc * 512 : (dc + 1) * 512],
                                        start=(hp == 0),
                                        stop=(hp == 1),
                                    )
                                oe = work.tile([128, 512], bf16, tag="oevict", bufs=3)
                                if qc % 2 == 0:
                                    nc.vector.tensor_copy(out=oe[:], in_=ps_o[:])
                                else:
                                    nc.scalar.copy(out=oe[:], in_=ps_o[:])
                                c0 = (dc - dgrp[0]) * 512
                                nc.sync.dma_start(
                                    out=ar_in[qc * 128 : (qc + 1) * 128, c0 : c0 + 512],
                                    in_=oe[:],
                                )
                        nc.gpsimd.collective_compute(
                            "ReduceScatter",
                            OP.add,
                            replica_groups=GROUPS,
                            ins=[ar_in[:, :].opt()],
                            outs=[ar_out[:, :].opt()],
                        )
                        outs.append((dgrp, ar_out))
                    tails[q] = outs

                ers = {}

                def quarter_ln(q):
                    rsl = slice(q * 128, (q + 1) * 128)
                    y = work.tile([128, D], f32, tag="y", bufs=2)
                    er = ers.pop(q)
                    stats = work.tile([128, 2, nc.vector.BN_STATS_DIM], f32, tag="stats")
                    rsb = work.tile([128, D], bf16, tag="rsb", bufs=2)
                    for dgrp, ar_out in tails.pop(q):
                        c0 = dgrp[0] * 512
                        dw = 512 * len(dgrp)
                        for ch in range(0, dw, 256):
                            nc.sync.dma_start(
                                out=rsb[:, c0 + ch : c0 + ch + 256],
                                in_=ar_out[:, ch : ch + 256],
                            )
                        for dc in dgrp:
                            hsl = slice(dc * 512, (dc + 1) * 512)
                            nc.vector.tensor_tensor(y[:, hsl], er[:, hsl], rsb[:, hsl], OP.add)
                            nc.vector.bn_stats(out=stats[:, dc, :], in_=y[:, hsl])
                    mv = work.tile([128, nc.vector.BN_AGGR_DIM], f32, tag="mv")
                    nc.vector.bn_aggr(out=mv[:], in_=stats[:])
                    rstd = work.tile([128, 1], f32, tag="rstd")
                    nc.scalar.activation(
                        out=rstd[:], in_=mv[:, 1:2], func=AF.Sqrt, bias=eps_sb[:], scale=1.0
                    )
                    nc.vector.reciprocal(rstd[:], rstd[:])
                    yn = work.tile([128, D], bf16, tag="yn", bufs=2)
                    nc.vector.tensor_scalar(
                        yn[:], y[:], mv[:, 0:1], rstd[:], OP.subtract, OP.mult
                    )
                    o = work.tile([128, D], bf16, tag="obf", bufs=2)
                    nc.vector.tensor_tensor(o[:], yn[:], gammabc[:], OP.mult)
                    nc.vector.tensor_tensor(o[:], o[:], betabc[:], OP.add)
                    for c in range(4):
                        nc.sync.dma_start(
                            out=out_d[rsl, c * 256 : (c + 1) * 256],
                            in_=o[:, c * 256 : (c + 1) * 256],
                        )

                def finish(pu):
                    normalize_evict(pu)
                    if pu["h"] == 3:
                        quarter_rs(pu["q"])

                # per-kb mask DMAs spread the 2MB load across all 16 DMA
                # engines (a single strided DMA serializes on one engine)
                def load_mask(q):
                    mq = work.tile([128, 16, 512], bf16, name="mq", tag="maskq")
                    for kb in range(16):
                        nc.sync.dma_start(
                            out=mq[:, kb, :],
                            in_=maskT_d[kb * 128 : (kb + 1) * 128, q * 512 : (q + 1) * 512],
                        )
                    return mq

                def load_er(q):
                    er = erp.tile([128, D], bf16, tag="er")
                    for c in range(4):
                        nc.sync.dma_start(
                            out=er[:, c * 256 : (c + 1) * 256],
                            in_=embres_d[q * 128 : (q + 1) * 128, c * 256 : (c + 1) * 256],
                        )
                    ers[q] = er

                units = []
                mqs = {0: load_mask(0)}
                for q in range(4):
                    load_er(q)
                for quarter in range(4):
                    qoff = quarter * 512
                    for h in range(4):
                        if h == 0 and quarter + 1 < 4:
                            mqs[quarter + 1] = load_mask(quarter + 1)
                        mq = mqs[quarter]
                        probs = probsp.tile([128, 16, 512], bf16, tag="probs")
                        unit = {"q": quarter, "h": h, "probs": probs, "ps_x": None}
                        for j in range(8):  # kb pairs
                            ps_s = psS.tile([128, 2, 512], f32, tag="score")
                            nc.tensor.matmul(
                                ps_s[:, 0, :],
                                k_sb[0:64, h, j, :],
                                q2_sb[0:64, h, qoff : qoff + 512],
                                start=True,
                                stop=True,
                                tile_position=(0, 0),
                            )
                            nc.tensor.matmul(
                                ps_s[:, 1, :],
                                k_sb[64:128, h, j, :],
                                q2_sb[64:128, h, qoff : qoff + 512],
                                start=True,
                                stop=True,
                                tile_position=(64, 0),
                            )
                            if units:
                                pv_mms(units[-1], 2 * j, 2 * j + 2)
                            if j < N_SCH:
                                # fused softmax+mask: bf16 bits of exp(s/8)
                                # built as (i16)(s*(184.665/8) + maskc2);
                                # masked lanes saturate to 0x8000 = -0.0
                                nc.vector.scalar_tensor_tensor(
                                    out=probs[:, 2 * j : 2 * j + 2, :].bitcast(i16dt),
                                    in0=ps_s[:, :, :],
                                    scalar=23.08312606 / (WS * WS),
                                    in1=mq[:, 2 * j : 2 * j + 2, :],
                                    op0=OP.mult,
                                    op1=OP.add,
                                )
                            else:
                                nc.scalar.activation(
                                    out=probs[:, 2 * j : 2 * j + 2, :],
                                    in_=ps_s[:, :, :],
                                    func=AF.Exp,
                                    scale=0.125 / (WS * WS),
                                )
                            if j == 3:  # mask for the ACT-path kbs of jobs 2,3
                                nc.vector.tensor_tensor(
                                    probs[:, 2 * N_SCH : 8, :],
                                    probs[:, 2 * N_SCH : 8, :],
                                    mq[:, 2 * N_SCH : 8, :],
                                    OP.mult,
                                )
                            elif j == 7:
                                nc.vector.tensor_tensor(
                                    probs[:, 8:16, :],
                                    probs[:, 8:16, :],
                                    mq[:, 8:16, :],
                                    OP.mult,
                                )
                            if j == 1 and len(units) >= 2:
                                finish_a(units[-2])
                            if j == 5 and len(units) >= 2:
                                finish(units[-2])
                            if j == 5 and quarter == 3 and h == 3:
                                quarter_ln(0)
                        units.append(unit)
                        if quarter == 0 and h == 0:
                            emit_v()
                        if debug and quarter == 0 and h == 1:
                            nc.sync.dma_start(out=dbg_p0_d[:, :, :], in_=units[0]["probs"][:, :, :])
                        if debug and quarter == 0 and h == 3:
                            nc.sync.dma_start(out=dbg_mq_d[:, :, :], in_=mqs[0][:, :, :])
                # tail: issue RS(3) first, then run all LNs (collective-
                # coupled work happens only here, so a late-starting peer
                # can never stall the mid-stream engine FIFOs)
                finish_a(units[-2])
                finish(units[-2])
                pv_mms(units[-1], 0, 16)
                finish_a(units[-1])
                finish(units[-1])
                for q in range(1, 4):
                    quarter_ln(q)
                if debug:
                    nc.sync.dma_start(out=dbg_q2_d[:, :, :], in_=q2_sb[:, :, :])
                    nc.sync.dma_start(out=dbg_k_d[:, :, :, :], in_=k_sb[:, :, :, :])
                    nc.sync.dma_start(out=dbg_v_d[:, :, :, :], in_=v_sb[:, :, :, :])
                    nc.sync.dma_start(out=dbg_xT_d[:, :, :], in_=xT_sb[:, :, :])

    nc.compile()
    return nc


def _prep_inputs(embeddings, attention_mask, W_qkv, b_qkv, W_out, b_out, ln_gamma, ln_beta):
    emb = np.asarray(embeddings, dtype=np.float32)
    mask = np.asarray(attention_mask)
    W_qkv = np.asarray(W_qkv, dtype=np.float32)
    b_qkv = np.asarray(b_qkv, dtype=np.float32)
    W_out = np.asarray(W_out, dtype=np.float32)
    b_out = np.asarray(b_out, dtype=np.float32)
    gamma = np.asarray(ln_gamma, dtype=np.float32).reshape(1, D).astype(ml_dtypes.bfloat16)
    beta = np.asarray(ln_beta, dtype=np.float32).reshape(1, D).astype(ml_dtypes.bfloat16)

    in_maps = []
    for c in range(NCORES):
        b = c // G
        g = c % G
        hs = g * HL * A  # 256g
        def dr_interleave(w):
            # [D, C] -> fp8 [128, (j ktp C)]: row 256*ktp + 128*j + p
            c = w.shape[1]
            r = w.reshape(4, 2, 128, c).transpose(2, 1, 0, 3)  # [p, j, ktp, c]
            return np.ascontiguousarray(r.reshape(128, 8 * c).astype(ml_dtypes.float8_e4m3fn))

        embT = dr_interleave(np.ascontiguousarray(emb[b].T))  # [D, S] interleaved
        # keys in the first N_SCH kb-pairs carry the Schraudolph additive
        # constants (16256 live / -65536 masked); the rest carry 1/0.
        maskT = np.ascontiguousarray(mask[b].T).astype(np.float32)
        ksch = 256 * N_SCH
        maskT[:ksch] = np.where(maskT[:ksch] > 0, 16256.0, -65536.0)
        maskT = maskT.astype(ml_dtypes.bfloat16)
        wqk = dr_interleave(
            np.concatenate(
                [W_qkv[:, hs : hs + 256], W_qkv[:, D + hs : D + hs + 256]], axis=1
            )
            * WS
        )
        wv = dr_interleave(W_qkv[:, 2 * D + hs : 2 * D + hs + 256] * WS)
        bqk = np.concatenate([b_qkv[hs : hs + 256], b_qkv[D + hs : D + hs + 256]]) * WS
        bqk = np.ascontiguousarray(bqk.reshape(4, 128).T)
        bv = np.ascontiguousarray(
            (b_qkv[2 * D + hs : 2 * D + hs + 256] * WS).reshape(1, 256)
        ).astype(ml_dtypes.bfloat16)
        wout = np.ascontiguousarray(
            W_out[hs : hs + 256, :].reshape(2, 128, D).transpose(1, 0, 2)
        ).astype(ml_dtypes.bfloat16)
        embres = np.concatenate(
            [emb[b, 512 * q + 128 * g : 512 * q + 128 * g + 128, :] for q in range(4)],
            axis=0,
        ) + b_out.reshape(1, D)
        in_maps.append(
            {
                "embT": embT,
                "embres": np.ascontiguousarray(embres.astype(ml_dtypes.bfloat16)),
                "maskT": maskT,
                "wqk": wqk,
                "wv": wv,
                "bqk": bqk,
                "bv": bv,
                "onesb": np.ones((1, 128), dtype=ml_dtypes.bfloat16),
                "wout": wout,
                "gamma": gamma,
                "beta": beta,
            }
        )
    return in_maps


def _run(inputs, trace=False, **kw):
    if "nc" not in _CACHE:
        _CACHE["nc"] = _build()
    nc = _CACHE["nc"]
    in_maps = _prep_inputs(**inputs)
    res = run_bass_kernel_spmd(nc, in_maps, list(range(NCORES)), trace=trace, **kw)
    out = np.empty((B, S, D), dtype=np.float32)
    for c in range(NCORES):
        b, g = c // G, c % G
        for q in range(4):
            out[b, 512 * q + 128 * g : 512 * q + 128 * g + 128, :] = (
                res.results[c]["out"][128 * q : 128 * (q + 1), :].astype(np.float32)
            )
    return out, res


def kernel(**inputs):
    out, _ = _run(inputs, trace=False)
    return out
